# revision 14
# baseline (speedup 1.0000x reference)
"""Multi-head attention (B=2, N=2048, C=1024, H=16, D=64) on 8 Trainium2 cores.

Sharding: core c handles batch b=c//4 and heads [4r, 4r+4) where r=c%4.
After per-head attention, AllToAll collectives redistribute the attention
output from head-sharded to sequence-sharded; core g computes the output
projection for rows [g*256, (g+1)*256) of both batches.

Design notes:
- q/k are computed directly transposed ([d, n] layout, two heads stacked per
  128-partition tile) with the weight matrix as the stationary operand; no PE
  transposes, and LayerNorm scale/bias become per-partition scalars.
- LayerNorm stats are PE matmuls against a 1/64 block-selector; rstd =
  1/sqrt(var+eps) via ACT Sqrt + DVE reciprocal; per-column normalization is
  applied with two bf16 DVE tensor_tensor ops against DMA-broadcast rows.
- Stage B softmax exp is split between the scalar engine (true exp) and the
  vector engine (Schraudolph exp2: bits = round(s*a+b) stored int16, viewed
  bf16). The split is per (pair, ih, head) unit so each softmax sum uses one
  engine consistently. Exp runs as two 512-wide calls per tile so the next
  tile's score matmuls can overwrite the already-consumed half (range WAR).
- Collectives are per (pair, head): 4 smaller AllToAlls instead of 2.
"""
import os
import numpy as np

B, N, C = 2, 2048, 1024
H, D = 16, 64
LN_EPS = 1e-6
N_CORES = 8
IH = 1024        # i-half width in the attention stage
NCH = 4          # stage-A n-chunks (512 each)

EXP_A = float(128.0 / np.log(2.0) * 0.125)
EXP_B = float(127.0 * 128.0)

DVE_FULL = {(0, 0, 1), (0, 1, 1), (1, 0, 1)}
DVE_SPLIT = (1, 1, 1)

_CACHE = {}


def _install_trace_shim():
    """Recreate the missing antenv.axon_hooks module so trace=True works."""
    import sys, types
    if "antenv.axon_hooks" in sys.modules:
        return
    try:
        import antenv
        mod = types.ModuleType("antenv.axon_hooks")
        mod._hook = None
        mod.set_axon_ntff_profile_hook = lambda h: setattr(mod, "_hook", h)
        mod.get_axon_ntff_profile_hook = lambda: mod._hook
        sys.modules["antenv.axon_hooks"] = mod
        antenv.axon_hooks = mod
        from trn_agent_boot.trn_boot import _ntff_profile_via_ctypes
        mod._hook = _ntff_profile_via_ctypes("/opt/axon/libaxon_pjrt.so")
    except Exception:
        pass


def _build(general):
    import concourse.bacc as bacc
    import concourse.bass as bass
    import concourse.tile as tile
    from concourse import mybir
    from contextlib import ExitStack

    f32 = mybir.dt.float32
    bf16 = mybir.dt.bfloat16
    i16 = mybir.dt.int16
    AF = mybir.ActivationFunctionType
    OP = mybir.AluOpType

    AP = bass.AP
    nc = bacc.Bacc("TRN2", target_bir_lowering=False, debug=False,
                   num_devices=N_CORES)

    # ---- DRAM I/O ----
    xT_d = nc.dram_tensor("xT", [C, N], bf16, kind="ExternalInput")
    wq_d = nc.dram_tensor("wq", [C, 2, 128], bf16, kind="ExternalInput")
    wk_d = nc.dram_tensor("wk", [C, 2, 128], bf16, kind="ExternalInput")
    wv_d = nc.dram_tensor("wv", [C, 256], bf16, kind="ExternalInput")
    wproj_d = nc.dram_tensor("wproj", [C, C], bf16, kind="ExternalInput")
    bq_d = nc.dram_tensor("bq", [2, 128], f32, kind="ExternalInput")
    bk_d = nc.dram_tensor("bk", [2, 128], f32, kind="ExternalInput")
    bv_d = nc.dram_tensor("bv", [256], f32, kind="ExternalInput")
    bproj_d = nc.dram_tensor("bproj", [C], f32, kind="ExternalInput")
    L_d = nc.dram_tensor("lnL", [128, 2], bf16, kind="ExternalInput")
    gq_d = nc.dram_tensor("gq", [2, 128], f32, kind="ExternalInput")
    gk_d = nc.dram_tensor("gk", [2, 128], f32, kind="ExternalInput")
    hq_d = nc.dram_tensor("hq", [2, 128], f32, kind="ExternalInput")
    hk_d = nc.dram_tensor("hk", [2, 128], f32, kind="ExternalInput")
    out_d = nc.dram_tensor("out_part", [B, 256, C], f32, kind="ExternalOutput")

    # DRAM scratch: [tensor, head, kind, n] rows out; [tensor, kind, head, n] rm
    stat_d = nc.dram_tensor("stat_scratch", [4, 2, 2, N], f32).ap()
    rm_d = nc.dram_tensor("rm_scratch", [4, 2, 2, N], bf16).ap()
    z_d = nc.dram_tensor("z_scratch", [8, IH], f32).ap()
    zr_d = nc.dram_tensor("zr_scratch", [8, IH], bf16).ap()

    def row_bcast(src, parts, free):
        return AP(tensor=src.tensor, offset=src.offset, ap=[[0, parts], [1, free]])

    groups = [[0, 1, 2, 3, 4, 5, 6, 7]]

    with tile.TileContext(nc) as tc:
        with ExitStack() as ctx:
            g = ctx.enter_context(tc.tile_pool(name="globals", bufs=1))
            dram = ctx.enter_context(tc.tile_pool(name="dram", bufs=1, space="DRAM"))

            # ---- consolidated input DMAs, spread across queues ----
            wv_sb = g.tile([128, 8, 256], bf16, tag="wv")
            wq_sb = g.tile([128, 2, 8, 128], bf16, tag="wq")
            wk_sb = g.tile([128, 2, 8, 128], bf16, tag="wk")
            nc.scalar.dma_start(out=wv_sb,
                                in_=wv_d.ap().rearrange("(kc p) c -> p kc c", p=128))
            nc.scalar.dma_start(out=wq_sb,
                                in_=wq_d.ap().rearrange("(kc p) r c -> p r kc c", p=128))
            nc.scalar.dma_start(out=wk_sb,
                                in_=wk_d.ap().rearrange("(kc p) r c -> p r kc c", p=128))

            xT = g.tile([128, 8, N], bf16, tag="xT")
            xa = xT_d.ap()
            for nw in range(4):
                nc.sync.dma_start(
                    out=xT[:, :, nw * 512:(nw + 1) * 512],
                    in_=AP(tensor=xa.tensor, offset=nw * 512,
                           ap=[[N, 128], [128 * N, 8], [1, 512]]))

            L_sb = g.tile([128, 2], bf16, tag="lnL")
            bq_sb = g.tile([128, 2], f32, tag="bq")
            bk_sb = g.tile([128, 2], f32, tag="bk")
            bv_bc = g.tile([128, 256], f32, tag="bv")
            bproj_bc = g.tile([128, C], f32, tag="bproj")
            eps_t = g.tile([128, 1], f32, tag="eps")
            nc.vector.memset(eps_t, LN_EPS)
            nc.gpsimd.dma_start(out=L_sb, in_=L_d.ap())
            nc.gpsimd.dma_start(out=bq_sb, in_=bq_d.ap().rearrange("r x -> x r"))
            nc.gpsimd.dma_start(out=bk_sb, in_=bk_d.ap().rearrange("r x -> x r"))
            if general:
                gq_sb = g.tile([128, 2], f32, tag="gq")
                gk_sb = g.tile([128, 2], f32, tag="gk")
                hq_sb = g.tile([128, 2], f32, tag="hq")
                hk_sb = g.tile([128, 2], f32, tag="hk")
                nc.gpsimd.dma_start(out=gq_sb, in_=gq_d.ap().rearrange("r x -> x r"))
                nc.gpsimd.dma_start(out=gk_sb, in_=gk_d.ap().rearrange("r x -> x r"))
                nc.gpsimd.dma_start(out=hq_sb, in_=hq_d.ap().rearrange("r x -> x r"))
                nc.gpsimd.dma_start(out=hk_sb, in_=hk_d.ap().rearrange("r x -> x r"))
            nc.gpsimd.dma_start(out=bv_bc, in_=row_bcast(bv_d.ap(), 128, 256))
            nc.gpsimd.dma_start(out=bproj_bc, in_=row_bcast(bproj_d.ap(), 128, C))

            wp_sb = g.tile([128, 8, C], bf16, tag="wp_sb")
            nc.gpsimd.dma_start(out=wp_sb,
                                in_=wproj_d.ap().rearrange("(kc p) n -> p kc n", p=128))

            # ---- persistent activations ----
            q2 = g.tile([128, 2, N], bf16, tag="q2")
            k2 = g.tile([128, 2, N], bf16, tag="k2")
            v_all = g.tile([128, 16, 4, D + 1], bf16, tag="v_all")
            ones_t = g.tile([128, 16, 4, 1], f32, tag="ones_t")
            nc.vector.memset(ones_t, 1.0)
            nc.vector.tensor_copy(out=v_all[:, :, :, D:D + 1], in_=ones_t)

            # per (pair, hp) collective tensors
            cc_in = [dram.tile([8, 64, 256], bf16, name=f"cc_in{u}") for u in range(4)]
            cc_out = [dram.tile([8, 64, 256], bf16, name=f"cc_out{u}") for u in range(4)]

            # ================= Stage A =================
            with ExitStack() as actx:
                sa = actx.enter_context(tc.tile_pool(name="stageA", bufs=2))
                sqp = actx.enter_context(tc.tile_pool(name="sq_pool", bufs=3))
                rmp = actx.enter_context(tc.tile_pool(name="rm_pool", bufs=2))
                stp = actx.enter_context(tc.tile_pool(name="stats", bufs=2))
                psQ = actx.enter_context(tc.tile_pool(name="psQ", bufs=2, space="PSUM"))
                psV = actx.enter_context(tc.tile_pool(name="psV", bufs=2, space="PSUM"))
                psS = actx.enter_context(tc.tile_pool(name="psS", bufs=1, space="PSUM"))

                for nt in range(16):
                    ps_v = psV.tile([128, 256], f32, tag="ps_v", name=f"ps_v{nt}")
                    for kc in range(8):
                        nc.tensor.matmul(ps_v, xT[:, kc, nt * 128:(nt + 1) * 128],
                                         wv_sb[:, kc, :], start=(kc == 0), stop=(kc == 7))
                    nc.vector.tensor_tensor(
                        out=v_all[:, nt, :, 0:D],
                        in0=ps_v.rearrange("p (h d) -> p h d", h=4),
                        in1=bv_bc.rearrange("p (h d) -> p h d", h=4),
                        op=OP.add)

                # tensors: (kind, pair): 0=q,1=k
                tensors = [(0, 0), (1, 0), (0, 1), (1, 1)]
                tmp_tiles = {}
                sq_tiles = {}
                st_ps = {}

                def emit_chunk(ti, ch):
                    kind, pair = tensors[ti]
                    w_sb = wq_sb if kind == 0 else wk_sb
                    b_sb = bq_sb if kind == 0 else bk_sb
                    nsl = slice(ch * 512, (ch + 1) * 512)
                    if ch == 0:
                        tmp_tiles[ti] = sa.tile([128, N], bf16, tag="qktmp", name=f"tmp{ti}")
                    tmp = tmp_tiles[ti]
                    ps_t = psQ.tile([128, 512], f32, tag="ps_t", name=f"ps_t{ti}_{ch}")
                    for kc in range(8):
                        nc.tensor.matmul(ps_t, w_sb[:, pair, kc, :], xT[:, kc, nsl],
                                         start=(kc == 0), stop=(kc == 7))
                    nc.scalar.activation(out=tmp[:, nsl], in_=ps_t, func=AF.Identity,
                                         bias=b_sb[:, pair:pair + 1], scale=1.0)
                    sq = sqp.tile([128, 512], bf16, tag="sq", name=f"sq{ti}_{ch}")
                    nc.vector.tensor_tensor(out=sq, in0=tmp[:, nsl], in1=tmp[:, nsl],
                                            op=OP.mult)
                    sq_tiles[(ti, ch)] = sq

                def emit_stats(ti, ch):
                    kind, pair = tensors[ti]
                    nsl = slice(ch * 512, (ch + 1) * 512)
                    tmp = tmp_tiles[ti]
                    if ch == 0:
                        st_ps[ti] = stp.tile([2, 2, N], f32, tag="st_rows", name=f"strow{ti}")
                    mu_rows = st_ps[ti]
                    p_r = psS.tile([2, 512], f32, tag="st_raw", name=f"st_r{ti}_{ch}")
                    p_s = psS.tile([2, 512], f32, tag="st_sq", name=f"st_s{ti}_{ch}")
                    nc.tensor.matmul(p_r, L_sb, tmp[:, nsl], start=True, stop=True)
                    nc.tensor.matmul(p_s, L_sb, sq_tiles.pop((ti, ch)), start=True, stop=True)
                    nc.scalar.activation(out=mu_rows[:, 0, nsl], in_=p_r, func=AF.Copy)
                    nc.scalar.activation(out=mu_rows[:, 1, nsl], in_=p_s, func=AF.Copy)

                def emit_post(ti):
                    kind, pair = tensors[ti]
                    mu_rows = st_ps.pop(ti)
                    tmp = tmp_tiles[ti]
                    nc.gpsimd.dma_start(out=stat_d[ti], in_=mu_rows)
                    st_t = stp.tile([128, 2, 2, 16], f32, tag="st_t", name=f"st_t{ti}")
                    for kd in range(2):
                        nc.gpsimd.dma_start(
                            out=st_t[:, kd],
                            in_=stat_d[ti, :, kd, :].rearrange("h (p i) -> p h i", p=128))
                    mu_t = st_t[:, 0]
                    m2_t = st_t[:, 1]
                    musq = stp.tile([128, 2, 16], f32, tag="musq", name=f"musq{ti}")
                    nc.vector.tensor_tensor(out=musq, in0=mu_t, in1=mu_t, op=OP.mult)
                    var = stp.tile([128, 2, 16], f32, tag="var", name=f"var{ti}")
                    nc.vector.tensor_tensor(out=var, in0=m2_t, in1=musq, op=OP.subtract)
                    sd = stp.tile([128, 2, 16], f32, tag="sd", name=f"sd{ti}")
                    nc.scalar.activation(out=sd, in_=var, func=AF.Sqrt, bias=eps_t)
                    rstd = stp.tile([128, 2, 16], f32, tag="rstd", name=f"rstd{ti}")
                    nc.vector.reciprocal(out=rstd, in_=sd)
                    mhat = stp.tile([128, 2, 16], f32, tag="mhat", name=f"mhat{ti}")
                    nc.vector.tensor_tensor(out=mhat, in0=mu_t, in1=rstd, op=OP.mult)
                    rm_bf = stp.tile([128, 2, 2, 16], bf16, tag="rm_bf", name=f"rm_bf{ti}")
                    nc.vector.tensor_copy(out=rm_bf[:, 0], in_=rstd)
                    nc.vector.tensor_copy(out=rm_bf[:, 1], in_=mhat)
                    for kd in range(2):
                        nc.gpsimd.dma_start(
                            out=rm_d[ti, kd].rearrange("h (p i) -> p h i", p=128),
                            in_=rm_bf[:, kd])
                    r_sb = rmp.tile([128, N], bf16, tag="r_sb", name=f"r_sb{ti}")
                    m_sb = rmp.tile([128, N], bf16, tag="m_sb", name=f"m_sb{ti}")
                    for hh in range(2):
                        nc.gpsimd.dma_start(out=r_sb[hh * 64:(hh + 1) * 64, :],
                                            in_=row_bcast(rm_d[ti, 0, hh], 64, N))
                        nc.gpsimd.dma_start(out=m_sb[hh * 64:(hh + 1) * 64, :],
                                            in_=row_bcast(rm_d[ti, 1, hh], 64, N))
                    dest = q2 if kind == 0 else k2
                    gg = (gq_sb if kind == 0 else gk_sb) if general else None
                    hh_b = (hq_sb if kind == 0 else hk_sb) if general else None
                    for ch in range(NCH):
                        nsl = slice(ch * 512, (ch + 1) * 512)
                        t1 = sqp.tile([128, 512], bf16, tag="t1", name=f"t1_{ti}_{ch}")
                        nc.vector.tensor_tensor(out=t1, in0=tmp[:, nsl], in1=r_sb[:, nsl],
                                                op=OP.mult)
                        if general:
                            t2 = sqp.tile([128, 512], bf16, tag="t2", name=f"t2_{ti}_{ch}")
                            nc.vector.tensor_tensor(out=t2, in0=t1, in1=m_sb[:, nsl],
                                                    op=OP.subtract)
                            nc.vector.tensor_scalar(
                                out=dest[:, pair, nsl], in0=t2,
                                scalar1=gg[:, pair:pair + 1], scalar2=hh_b[:, pair:pair + 1],
                                op0=OP.mult, op1=OP.add)
                        else:
                            nc.vector.tensor_tensor(out=dest[:, pair, nsl], in0=t1,
                                                    in1=m_sb[:, nsl], op=OP.subtract)

                # software-pipelined emission: stats lag chunks by one slot,
                # post-chain for tensor ti emitted right after its last stats.
                events = []
                pend = None
                for ti in range(4):
                    for ch in range(NCH):
                        events.append(("chunk", ti, ch))
                        if pend is not None:
                            events.append(("stat",) + pend)
                            if pend[1] == NCH - 1:
                                events.append(("post", pend[0]))
                        pend = (ti, ch)
                events.append(("stat",) + pend)
                events.append(("post", pend[0]))
                for ev in events:
                    if ev[0] == "chunk":
                        emit_chunk(ev[1], ev[2])
                    elif ev[0] == "stat":
                        emit_stats(ev[1], ev[2])
                    else:
                        emit_post(ev[1])

            # ================= Stage B + C =================
            atp = ctx.enter_context(tc.tile_pool(name="at_pool", bufs=1))
            at_tiles = {}

            def emit_at_loads(pair):
                # at_t tiles for stage-C chunks that read this pair's heads
                for kc in [k for k in range(8) if (2 * k) % 4 // 2 == pair]:
                    for bb in range(B):
                        at_t = atp.tile([128, 256], bf16, tag=f"at{kc}_{bb}")
                        at_tiles[(kc, bb)] = at_t
                        for half, gh in enumerate((2 * kc, 2 * kc + 1)):
                            lh = gh % 4
                            u = 2 * (lh // 2) + (lh % 2)
                            nc.sync.dma_start(
                                out=at_t[half * 64:(half + 1) * 64, :],
                                in_=cc_out[u][4 * bb + gh // 4, :, :])

            with ExitStack() as bctx:
                pss = bctx.enter_context(tc.tile_pool(name="psSc", bufs=1, space="PSUM"))
                pso = bctx.enter_context(tc.tile_pool(name="psO", bufs=1, space="PSUM"))
                ptp = bctx.enter_context(tc.tile_pool(name="pt_pool", bufs=6))
                nrm = bctx.enter_context(tc.tile_pool(name="nrm", bufs=3))

                for pair in range(2):
                    for ih in range(2):
                        ps_o = {}
                        for hp in range(2):
                            ps_o[hp] = pso.tile([65, IH], f32, tag=f"ps_o{hp}",
                                                name=f"ps_o{pair}_{ih}_{hp}")
                        for jt in range(16):
                            pts = {}
                            ps_s = {}
                            for hp in range(2):
                                ps_s[hp] = pss.tile([128, IH], f32, tag=f"ps_s{hp}",
                                                    name=f"ps_s{pair}_{ih}_{hp}_{jt}")
                            for icc in range(2):
                                for hp in range(2):
                                    po = hp * 64
                                    nc.tensor.matmul(
                                        ps_s[hp][:, icc * 512:(icc + 1) * 512],
                                        k2[po:po + 64, pair, jt * 128:(jt + 1) * 128],
                                        q2[po:po + 64, pair,
                                           ih * IH + icc * 512: ih * IH + (icc + 1) * 512],
                                        start=True, stop=True)
                            for hp in range(2):
                                pt = ptp.tile([128, IH], bf16, tag=f"pt{hp}",
                                              name=f"pt{pair}_{ih}_{hp}_{jt}")
                                unit = (pair, ih, hp)
                                if general:
                                    mode = "act"
                                elif unit in DVE_FULL:
                                    mode = "dve"
                                elif unit == DVE_SPLIT:
                                    mode = "split"
                                else:
                                    mode = "act"
                                for icc in range(2):
                                    csl = slice(icc * 512, (icc + 1) * 512)
                                    use_dve = (mode == "dve") or (mode == "split" and icc == 1)
                                    if use_dve:
                                        nc.vector.tensor_scalar(
                                            out=pt.bitcast(i16)[:, csl], in0=ps_s[hp][:, csl],
                                            scalar1=EXP_A, scalar2=EXP_B,
                                            op0=OP.mult, op1=OP.add)
                                    else:
                                        nc.scalar.activation(out=pt[:, csl],
                                                             in_=ps_s[hp][:, csl],
                                                             func=AF.Exp, scale=0.125)
                                pts[hp] = pt
                            for icc in range(2):
                                for hp in range(2):
                                    nc.tensor.matmul(
                                        ps_o[hp][:, icc * 512:(icc + 1) * 512],
                                        v_all[:, jt, 2 * pair + hp, :],
                                        pts[hp][:, icc * 512:(icc + 1) * 512],
                                        start=(jt == 0), stop=(jt == 15))

                        for hp in range(2):
                            h = 2 * pair + hp
                            slot = 2 * h + ih
                            z_sb = nrm.tile([1, IH], f32, tag="z_sb", name=f"z{slot}")
                            nc.scalar.activation(out=z_sb, in_=ps_o[hp][64:65, :], func=AF.Copy)
                            nc.sync.dma_start(out=z_d[slot:slot + 1, :], in_=z_sb)
                            zt = nrm.tile([128, 8], f32, tag="zt", name=f"zt{slot}")
                            nc.sync.dma_start(out=zt,
                                              in_=z_d[slot, :].rearrange("(p t) -> p t", p=128))
                            rt = nrm.tile([128, 8], f32, tag="rt", name=f"rt{slot}")
                            nc.vector.reciprocal(out=rt, in_=zt)
                            rt_bf = nrm.tile([128, 8], bf16, tag="rt_bf", name=f"rtb{slot}")
                            nc.vector.tensor_copy(out=rt_bf, in_=rt)
                            nc.sync.dma_start(out=zr_d[slot, :].rearrange("(p t) -> p t", p=128),
                                              in_=rt_bf)
                            r64 = nrm.tile([64, IH], bf16, tag="r64", name=f"r64_{slot}")
                            nc.sync.dma_start(out=r64, in_=row_bcast(zr_d[slot, :], 64, IH))
                            outT_t = nrm.tile([64, IH], bf16, tag="outT", name=f"oT{slot}")
                            nc.vector.tensor_tensor(out=outT_t, in0=ps_o[hp][0:64, :],
                                                    in1=r64, op=OP.mult)
                            nc.gpsimd.dma_start(
                                out=cc_in[2 * pair + hp][4 * ih:4 * ih + 4, :, :]
                                    .rearrange("s d i -> d s i"),
                                in_=outT_t.rearrange("d (s i) -> d s i", s=4))

                    for hp in range(2):
                        u = 2 * pair + hp
                        nc.gpsimd.collective_compute(
                            "AllToAll", mybir.AluOpType.bypass, replica_groups=groups,
                            ins=[cc_in[u].opt()], outs=[cc_out[u].opt()])
                    emit_at_loads(pair)

            with ExitStack() as cctx:
                psP = cctx.enter_context(tc.tile_pool(name="psP", bufs=1, space="PSUM"))
                oup = cctx.enter_context(tc.tile_pool(name="out_pool", bufs=3))

                ps_list = {}
                for bb in range(B):
                    for mt in range(2):
                        for nk in range(2):
                            ps_p = psP.tile([128, 512], f32, tag=f"ps_p{bb}{mt}{nk}")
                            ps_list[(bb, mt, nk)] = ps_p
                kc_order = [0, 2, 4, 6, 1, 3, 5, 7]
                for ki, kc in enumerate(kc_order):
                    wp_t = wp_sb[:, kc, :]
                    for bb in range(B):
                        at_t = at_tiles[(kc, bb)]
                        for mt in range(2):
                            for nk in range(2):
                                nc.tensor.matmul(
                                    ps_list[(bb, mt, nk)],
                                    at_t[:, mt * 128:(mt + 1) * 128],
                                    wp_t[:, nk * 512:(nk + 1) * 512],
                                    start=(ki == 0), stop=(ki == 7))
                for bb in range(B):
                    for mt in range(2):
                        o_sb = oup.tile([128, C], f32, tag="o_sb")
                        for nk in range(2):
                            nc.vector.tensor_tensor(
                                out=o_sb[:, nk * 512:(nk + 1) * 512],
                                in0=ps_list[(bb, mt, nk)],
                                in1=bproj_bc[:, nk * 512:(nk + 1) * 512],
                                op=OP.add)
                        nc.sync.dma_start(
                            out=out_d.ap()[bb, mt * 128:(mt + 1) * 128, :], in_=o_sb)

    nc.compile()
    return nc


def kernel(**inputs):
    from concourse.bass_utils import run_bass_kernel_spmd
    import ml_dtypes

    trace = os.environ.get("KERNEL_TRACE", "0") == "1"
    if trace:
        _install_trace_shim()

    bf = ml_dtypes.bfloat16

    x = np.asarray(inputs["x"], dtype=np.float32)
    w_qkv = np.asarray(inputs["w_qkv"], dtype=np.float32)
    b_qkv = np.asarray(inputs["b_qkv"], dtype=np.float32)
    w_proj = np.asarray(inputs["w_proj"], dtype=np.float32)
    b_proj = np.asarray(inputs["b_proj"], dtype=np.float32)
    q_scale = np.asarray(inputs["q_scale"], dtype=np.float32)
    q_bias = np.asarray(inputs["q_bias"], dtype=np.float32)
    k_scale = np.asarray(inputs["k_scale"], dtype=np.float32)
    k_bias = np.asarray(inputs["k_bias"], dtype=np.float32)

    general = not (np.all(q_scale == 1.0) and np.all(k_scale == 1.0)
                   and np.all(q_bias == 0.0) and np.all(k_bias == 0.0))

    key = "nc_gen" if general else "nc_fast"
    if key not in _CACHE:
        _CACHE[key] = _build(general)
    nc = _CACHE[key]

    L = np.zeros((128, 2), dtype=np.float32)
    L[0:64, 0] = 1.0 / 64.0
    L[64:128, 1] = 1.0 / 64.0

    wproj_m = np.ascontiguousarray(w_proj.astype(bf))

    in_maps = []
    for c in range(N_CORES):
        b, r = divmod(c, 4)
        base = 4 * r * D
        wq = np.ascontiguousarray(
            w_qkv[:, 0 * C + base: 0 * C + base + 256].reshape(C, 2, 128).astype(bf))
        wk = np.ascontiguousarray(
            w_qkv[:, 1 * C + base: 1 * C + base + 256].reshape(C, 2, 128).astype(bf))
        wv = np.ascontiguousarray(w_qkv[:, 2 * C + base: 2 * C + base + 256].astype(bf))
        bq = np.ascontiguousarray(b_qkv[0 * C + base: 0 * C + base + 256].reshape(2, 128))
        bk = np.ascontiguousarray(b_qkv[1 * C + base: 1 * C + base + 256].reshape(2, 128))
        bv = np.ascontiguousarray(b_qkv[2 * C + base: 2 * C + base + 256])
        m = {
            "xT": np.ascontiguousarray(x[b].T.astype(bf)),
            "wq": wq, "wk": wk, "wv": wv, "wproj": wproj_m,
            "bq": bq, "bk": bk, "bv": bv, "bproj": b_proj,
            "lnL": np.ascontiguousarray(L.astype(bf)),
        }
        if general:
            m["gq"] = np.ascontiguousarray(np.tile(q_scale, 2).reshape(2, 128))
            m["gk"] = np.ascontiguousarray(np.tile(k_scale, 2).reshape(2, 128))
            m["hq"] = np.ascontiguousarray(np.tile(q_bias, 2).reshape(2, 128))
            m["hk"] = np.ascontiguousarray(np.tile(k_bias, 2).reshape(2, 128))
        else:
            z2 = np.zeros((2, 128), dtype=np.float32)
            m["gq"] = z2; m["gk"] = z2; m["hq"] = z2; m["hk"] = z2
        in_maps.append(m)

    res = run_bass_kernel_spmd(nc, in_maps, core_ids=list(range(N_CORES)),
                               trace=trace)
    _CACHE["last_result"] = res

    out = np.empty((B, N, C), dtype=np.float32)
    for c in range(N_CORES):
        out[:, c * 256:(c + 1) * 256, :] = res.results[c]["out_part"]
    return out


# revision 15
# speedup vs baseline: 1.0378x; 1.0378x over previous
"""Multi-head attention (B=2, N=2048, C=1024, H=16, D=64) on 8 Trainium2 cores.

Sharding: core c handles batch b=c//4 and heads [4r, 4r+4) where r=c%4.
After per-head attention, AllToAll collectives redistribute the attention
output from head-sharded to sequence-sharded; core g computes the output
projection for rows [g*256, (g+1)*256) of both batches.

Design notes:
- q/k are computed directly transposed ([d, n] layout, two heads stacked per
  128-partition tile) with the weight matrix as the stationary operand; no PE
  transposes, and LayerNorm scale/bias become per-partition scalars.
- LayerNorm stats are PE matmuls against a 1/64 block-selector; rstd =
  1/sqrt(var+eps) via ACT Sqrt + DVE reciprocal; per-column normalization is
  applied with two bf16 DVE tensor_tensor ops against DMA-broadcast rows.
- Stage B softmax exp is split between the scalar engine (true exp) and the
  vector engine (Schraudolph exp2: bits = round(s*a+b) stored int16, viewed
  bf16). The split is per (pair, ih, head) unit so each softmax sum uses one
  engine consistently. Exp runs as two 512-wide calls per tile so the next
  tile's score matmuls can overwrite the already-consumed half (range WAR).
- Collectives are per (pair, head): 4 smaller AllToAlls instead of 2.
"""
import os
import numpy as np

B, N, C = 2, 2048, 1024
H, D = 16, 64
LN_EPS = 1e-6
N_CORES = 8
IH = 1024        # i-half width in the attention stage
NCH = 4          # stage-A n-chunks (512 each)

EXP_A = float(128.0 / np.log(2.0) * 0.125)
EXP_B = float(127.0 * 128.0)

DVE_FULL = {(0, 0, 1), (0, 1, 1), (1, 0, 1)}
DVE_SPLIT = (1, 1, 1)

_CACHE = {}


def _install_trace_shim():
    """Recreate the missing antenv.axon_hooks module so trace=True works."""
    import sys, types
    if "antenv.axon_hooks" in sys.modules:
        return
    try:
        import antenv
        mod = types.ModuleType("antenv.axon_hooks")
        mod._hook = None
        mod.set_axon_ntff_profile_hook = lambda h: setattr(mod, "_hook", h)
        mod.get_axon_ntff_profile_hook = lambda: mod._hook
        sys.modules["antenv.axon_hooks"] = mod
        antenv.axon_hooks = mod
        from trn_agent_boot.trn_boot import _ntff_profile_via_ctypes
        mod._hook = _ntff_profile_via_ctypes("/opt/axon/libaxon_pjrt.so")
    except Exception:
        pass


def _build(general):
    import concourse.bacc as bacc
    import concourse.bass as bass
    import concourse.tile as tile
    from concourse import mybir
    from contextlib import ExitStack

    f32 = mybir.dt.float32
    bf16 = mybir.dt.bfloat16
    i16 = mybir.dt.int16
    AF = mybir.ActivationFunctionType
    OP = mybir.AluOpType

    AP = bass.AP
    nc = bacc.Bacc("TRN2", target_bir_lowering=False, debug=False,
                   num_devices=N_CORES)

    # ---- DRAM I/O ----
    xT_d = nc.dram_tensor("xT", [C, N], bf16, kind="ExternalInput")
    wq_d = nc.dram_tensor("wq", [C, 2, 128], bf16, kind="ExternalInput")
    wk_d = nc.dram_tensor("wk", [C, 2, 128], bf16, kind="ExternalInput")
    wv_d = nc.dram_tensor("wv", [C, 256], bf16, kind="ExternalInput")
    wproj_d = nc.dram_tensor("wproj", [C, C], bf16, kind="ExternalInput")
    bq_d = nc.dram_tensor("bq", [2, 128], f32, kind="ExternalInput")
    bk_d = nc.dram_tensor("bk", [2, 128], f32, kind="ExternalInput")
    bv_d = nc.dram_tensor("bv", [256], f32, kind="ExternalInput")
    bproj_d = nc.dram_tensor("bproj", [C], f32, kind="ExternalInput")
    L_d = nc.dram_tensor("lnL", [128, 2], bf16, kind="ExternalInput")
    gq_d = nc.dram_tensor("gq", [2, 128], f32, kind="ExternalInput")
    gk_d = nc.dram_tensor("gk", [2, 128], f32, kind="ExternalInput")
    hq_d = nc.dram_tensor("hq", [2, 128], f32, kind="ExternalInput")
    hk_d = nc.dram_tensor("hk", [2, 128], f32, kind="ExternalInput")
    out_d = nc.dram_tensor("out_part", [B, 256, C], f32, kind="ExternalOutput")

    # DRAM scratch: [tensor, head, kind, n] rows out; [tensor, kind, head, n] rm
    stat_d = nc.dram_tensor("stat_scratch", [4, 2, 2, N], f32).ap()
    rm_d = nc.dram_tensor("rm_scratch", [4, 2, 2, N], bf16).ap()
    z_d = nc.dram_tensor("z_scratch", [8, IH], f32).ap()
    zr_d = nc.dram_tensor("zr_scratch", [8, IH], bf16).ap()

    def row_bcast(src, parts, free):
        return AP(tensor=src.tensor, offset=src.offset, ap=[[0, parts], [1, free]])

    groups = [[0, 1, 2, 3, 4, 5, 6, 7]]

    with tile.TileContext(nc) as tc:
        with ExitStack() as ctx:
            g = ctx.enter_context(tc.tile_pool(name="globals", bufs=1))
            dram = ctx.enter_context(tc.tile_pool(name="dram", bufs=1, space="DRAM"))

            # ---- consolidated input DMAs, spread across queues ----
            wv_sb = g.tile([128, 8, 256], bf16, tag="wv")
            wq_sb = g.tile([128, 2, 8, 128], bf16, tag="wq")
            wk_sb = g.tile([128, 2, 8, 128], bf16, tag="wk")
            nc.scalar.dma_start(out=wv_sb,
                                in_=wv_d.ap().rearrange("(kc p) c -> p kc c", p=128))
            nc.scalar.dma_start(out=wq_sb,
                                in_=wq_d.ap().rearrange("(kc p) r c -> p r kc c", p=128))
            nc.scalar.dma_start(out=wk_sb,
                                in_=wk_d.ap().rearrange("(kc p) r c -> p r kc c", p=128))

            xT = g.tile([128, 8, N], bf16, tag="xT")
            xa = xT_d.ap()
            for nw in range(4):
                nc.sync.dma_start(
                    out=xT[:, :, nw * 512:(nw + 1) * 512],
                    in_=AP(tensor=xa.tensor, offset=nw * 512,
                           ap=[[N, 128], [128 * N, 8], [1, 512]]))

            L_sb = g.tile([128, 2], bf16, tag="lnL")
            bq_sb = g.tile([128, 2], f32, tag="bq")
            bk_sb = g.tile([128, 2], f32, tag="bk")
            bv_bc = g.tile([128, 256], f32, tag="bv")
            bproj_bc = g.tile([128, C], f32, tag="bproj")
            eps_t = g.tile([128, 1], f32, tag="eps")
            nc.vector.memset(eps_t, LN_EPS)
            nc.gpsimd.dma_start(out=L_sb, in_=L_d.ap())
            nc.gpsimd.dma_start(out=bq_sb, in_=bq_d.ap().rearrange("r x -> x r"))
            nc.gpsimd.dma_start(out=bk_sb, in_=bk_d.ap().rearrange("r x -> x r"))
            if general:
                gq_sb = g.tile([128, 2], f32, tag="gq")
                gk_sb = g.tile([128, 2], f32, tag="gk")
                hq_sb = g.tile([128, 2], f32, tag="hq")
                hk_sb = g.tile([128, 2], f32, tag="hk")
                nc.gpsimd.dma_start(out=gq_sb, in_=gq_d.ap().rearrange("r x -> x r"))
                nc.gpsimd.dma_start(out=gk_sb, in_=gk_d.ap().rearrange("r x -> x r"))
                nc.gpsimd.dma_start(out=hq_sb, in_=hq_d.ap().rearrange("r x -> x r"))
                nc.gpsimd.dma_start(out=hk_sb, in_=hk_d.ap().rearrange("r x -> x r"))
            nc.gpsimd.dma_start(out=bv_bc, in_=row_bcast(bv_d.ap(), 128, 256))
            nc.gpsimd.dma_start(out=bproj_bc, in_=row_bcast(bproj_d.ap(), 128, C))

            wp_sb = g.tile([128, 8, C], bf16, tag="wp_sb")
            nc.scalar.dma_start(out=wp_sb,
                                in_=wproj_d.ap().rearrange("(kc p) n -> p kc n", p=128))

            # ---- persistent activations ----
            q2 = g.tile([128, 2, N], bf16, tag="q2")
            k2 = g.tile([128, 2, N], bf16, tag="k2")
            v_all = g.tile([128, 16, 4, D + 1], bf16, tag="v_all")
            ones_t = g.tile([128, 16, 4, 1], f32, tag="ones_t")
            nc.vector.memset(ones_t, 1.0)
            nc.vector.tensor_copy(out=v_all[:, :, :, D:D + 1], in_=ones_t)

            # per-pair collective tensors
            cc_in = [dram.tile([8, 128, 256], bf16, name=f"cc_in{p}") for p in range(2)]
            cc_out = [dram.tile([8, 128, 256], bf16, name=f"cc_out{p}") for p in range(2)]

            # ================= Stage A =================
            with ExitStack() as actx:
                sa = actx.enter_context(tc.tile_pool(name="stageA", bufs=2))
                sqp = actx.enter_context(tc.tile_pool(name="sq_pool", bufs=3))
                rmp = actx.enter_context(tc.tile_pool(name="rm_pool", bufs=2))
                stp = actx.enter_context(tc.tile_pool(name="stats", bufs=2))
                psQ = actx.enter_context(tc.tile_pool(name="psQ", bufs=2, space="PSUM"))
                psV = actx.enter_context(tc.tile_pool(name="psV", bufs=2, space="PSUM"))
                psS = actx.enter_context(tc.tile_pool(name="psS", bufs=1, space="PSUM"))

                def emit_v(nt):
                    ps_v = psV.tile([128, 256], f32, tag="ps_v", name=f"ps_v{nt}")
                    for kc in range(8):
                        nc.tensor.matmul(ps_v, xT[:, kc, nt * 128:(nt + 1) * 128],
                                         wv_sb[:, kc, :], start=(kc == 0), stop=(kc == 7))
                    nc.vector.tensor_tensor(
                        out=v_all[:, nt, :, 0:D],
                        in0=ps_v.rearrange("p (h d) -> p h d", h=4),
                        in1=bv_bc.rearrange("p (h d) -> p h d", h=4),
                        op=OP.add)

                # tensors: (kind, pair): 0=q,1=k
                tensors = [(0, 0), (1, 0), (0, 1), (1, 1)]
                tmp_tiles = {}
                sq_tiles = {}
                st_ps = {}

                def emit_chunk(ti, ch):
                    kind, pair = tensors[ti]
                    w_sb = wq_sb if kind == 0 else wk_sb
                    b_sb = bq_sb if kind == 0 else bk_sb
                    nsl = slice(ch * 512, (ch + 1) * 512)
                    if ch == 0:
                        tmp_tiles[ti] = sa.tile([128, N], bf16, tag="qktmp", name=f"tmp{ti}")
                    tmp = tmp_tiles[ti]
                    ps_t = psQ.tile([128, 512], f32, tag="ps_t", name=f"ps_t{ti}_{ch}")
                    for kc in range(8):
                        nc.tensor.matmul(ps_t, w_sb[:, pair, kc, :], xT[:, kc, nsl],
                                         start=(kc == 0), stop=(kc == 7))
                    nc.scalar.activation(out=tmp[:, nsl], in_=ps_t, func=AF.Identity,
                                         bias=b_sb[:, pair:pair + 1], scale=1.0)
                    sq = sqp.tile([128, 512], bf16, tag="sq", name=f"sq{ti}_{ch}")
                    nc.vector.tensor_tensor(out=sq, in0=tmp[:, nsl], in1=tmp[:, nsl],
                                            op=OP.mult)
                    sq_tiles[(ti, ch)] = sq

                def emit_stats(ti, ch):
                    kind, pair = tensors[ti]
                    nsl = slice(ch * 512, (ch + 1) * 512)
                    tmp = tmp_tiles[ti]
                    if ch == 0:
                        st_ps[ti] = stp.tile([2, 2, N], f32, tag="st_rows", name=f"strow{ti}")
                    mu_rows = st_ps[ti]
                    p_b = psS.tile([2, 1024], f32, tag="st_b", name=f"st_b{ti}_{ch}")
                    nc.tensor.matmul(p_b[:, 0:512], L_sb, tmp[:, nsl], start=True, stop=True)
                    nc.tensor.matmul(p_b[:, 512:1024], L_sb, sq_tiles.pop((ti, ch)),
                                     start=True, stop=True)
                    nc.scalar.activation(out=mu_rows[:, :, nsl],
                                         in_=p_b.rearrange("h (k n) -> h k n", k=2),
                                         func=AF.Copy)

                def emit_post(ti):
                    kind, pair = tensors[ti]
                    mu_rows = st_ps.pop(ti)
                    tmp = tmp_tiles[ti]
                    nc.sync.dma_start(out=stat_d[ti], in_=mu_rows)
                    st_t = stp.tile([128, 2, 2, 16], f32, tag="st_t", name=f"st_t{ti}")
                    for kd in range(2):
                        nc.sync.dma_start(
                            out=st_t[:, kd],
                            in_=stat_d[ti, :, kd, :].rearrange("h (p i) -> p h i", p=128))
                    mu_t = st_t[:, 0]
                    m2_t = st_t[:, 1]
                    musq = stp.tile([128, 2, 16], f32, tag="musq", name=f"musq{ti}")
                    nc.vector.tensor_tensor(out=musq, in0=mu_t, in1=mu_t, op=OP.mult)
                    var = stp.tile([128, 2, 16], f32, tag="var", name=f"var{ti}")
                    nc.vector.tensor_tensor(out=var, in0=m2_t, in1=musq, op=OP.subtract)
                    sd = stp.tile([128, 2, 16], f32, tag="sd", name=f"sd{ti}")
                    nc.scalar.activation(out=sd, in_=var, func=AF.Sqrt, bias=eps_t)
                    rstd = stp.tile([128, 2, 16], f32, tag="rstd", name=f"rstd{ti}")
                    nc.vector.reciprocal(out=rstd, in_=sd)
                    mhat = stp.tile([128, 2, 16], f32, tag="mhat", name=f"mhat{ti}")
                    nc.vector.tensor_tensor(out=mhat, in0=mu_t, in1=rstd, op=OP.mult)
                    rm_bf = stp.tile([128, 2, 2, 16], bf16, tag="rm_bf", name=f"rm_bf{ti}")
                    nc.vector.tensor_copy(out=rm_bf[:, 0], in_=rstd)
                    nc.vector.tensor_copy(out=rm_bf[:, 1], in_=mhat)
                    for kd in range(2):
                        nc.sync.dma_start(
                            out=rm_d[ti, kd].rearrange("h (p i) -> p h i", p=128),
                            in_=rm_bf[:, kd])
                    r_sb = rmp.tile([128, N], bf16, tag="r_sb", name=f"r_sb{ti}")
                    m_sb = rmp.tile([128, N], bf16, tag="m_sb", name=f"m_sb{ti}")
                    for hh in range(2):
                        nc.sync.dma_start(out=r_sb[hh * 64:(hh + 1) * 64, :],
                                            in_=row_bcast(rm_d[ti, 0, hh], 64, N))
                        nc.sync.dma_start(out=m_sb[hh * 64:(hh + 1) * 64, :],
                                            in_=row_bcast(rm_d[ti, 1, hh], 64, N))
                    dest = q2 if kind == 0 else k2
                    gg = (gq_sb if kind == 0 else gk_sb) if general else None
                    hh_b = (hq_sb if kind == 0 else hk_sb) if general else None
                    for ch in range(NCH):
                        nsl = slice(ch * 512, (ch + 1) * 512)
                        t1 = sqp.tile([128, 512], bf16, tag="t1", name=f"t1_{ti}_{ch}")
                        nc.vector.tensor_tensor(out=t1, in0=tmp[:, nsl], in1=r_sb[:, nsl],
                                                op=OP.mult)
                        if general:
                            t2 = sqp.tile([128, 512], bf16, tag="t2", name=f"t2_{ti}_{ch}")
                            nc.vector.tensor_tensor(out=t2, in0=t1, in1=m_sb[:, nsl],
                                                    op=OP.subtract)
                            nc.vector.tensor_scalar(
                                out=dest[:, pair, nsl], in0=t2,
                                scalar1=gg[:, pair:pair + 1], scalar2=hh_b[:, pair:pair + 1],
                                op0=OP.mult, op1=OP.add)
                        else:
                            nc.vector.tensor_tensor(out=dest[:, pair, nsl], in0=t1,
                                                    in1=m_sb[:, nsl], op=OP.subtract)

                # software-pipelined emission: v-tiles interleave with qk
                # chunks, stats lag chunks by one slot, post-chain for tensor
                # ti emitted right after its last stats.
                pend = None
                for slot in range(16):
                    emit_v(slot)
                    emit_chunk(slot // NCH, slot % NCH)
                    if pend is not None:
                        emit_stats(*pend)
                        if pend[1] == NCH - 1:
                            emit_post(pend[0])
                    pend = (slot // NCH, slot % NCH)
                emit_stats(*pend)
                emit_post(pend[0])

            # ================= Stage B + C =================
            atp = ctx.enter_context(tc.tile_pool(name="at_pool", bufs=1))
            at_tiles = {}

            def emit_at_loads(pair):
                # at_t tiles for stage-C chunks that read this pair's heads;
                # one DMA per (kc, head-half) covering both batches.
                for kc in [k for k in range(8) if k % 2 == pair]:
                    at_t = atp.tile([128, 2, 256], bf16, tag=f"at{kc}")
                    at_tiles[kc] = at_t
                    for half, gh in enumerate((2 * kc, 2 * kc + 1)):
                        lh = gh % 4
                        src = cc_out[lh // 2]
                        nc.gpsimd.dma_start(
                            out=at_t[half * 64:(half + 1) * 64, :, :],
                            in_=src[gh // 4::4, (lh % 2) * 64:(lh % 2 + 1) * 64, :]
                                .rearrange("b d i -> d b i"))

            with ExitStack() as bctx:
                pss = bctx.enter_context(tc.tile_pool(name="psSc", bufs=1, space="PSUM"))
                pso = bctx.enter_context(tc.tile_pool(name="psO", bufs=1, space="PSUM"))
                ptp = bctx.enter_context(tc.tile_pool(name="pt_pool", bufs=6))
                nrm = bctx.enter_context(tc.tile_pool(name="nrm", bufs=3))

                for pair in range(2):
                    for ih in range(2):
                        ps_o = {}
                        for hp in range(2):
                            ps_o[hp] = pso.tile([65, IH], f32, tag=f"ps_o{hp}",
                                                name=f"ps_o{pair}_{ih}_{hp}")
                        for jt in range(16):
                            pts = {}
                            ps_s = {}
                            for hp in range(2):
                                ps_s[hp] = pss.tile([128, IH], f32, tag=f"ps_s{hp}",
                                                    name=f"ps_s{pair}_{ih}_{hp}_{jt}")
                            for icc in range(2):
                                for hp in range(2):
                                    po = hp * 64
                                    nc.tensor.matmul(
                                        ps_s[hp][:, icc * 512:(icc + 1) * 512],
                                        k2[po:po + 64, pair, jt * 128:(jt + 1) * 128],
                                        q2[po:po + 64, pair,
                                           ih * IH + icc * 512: ih * IH + (icc + 1) * 512],
                                        start=True, stop=True)
                            for hp in range(2):
                                pt = ptp.tile([128, IH], bf16, tag=f"pt{hp}",
                                              name=f"pt{pair}_{ih}_{hp}_{jt}")
                                unit = (pair, ih, hp)
                                if general:
                                    mode = "act"
                                elif unit in DVE_FULL:
                                    mode = "dve"
                                elif unit == DVE_SPLIT:
                                    mode = "split"
                                else:
                                    mode = "act"
                                for icc in range(2):
                                    csl = slice(icc * 512, (icc + 1) * 512)
                                    use_dve = (mode == "dve") or (mode == "split" and icc == 1)
                                    if use_dve:
                                        nc.vector.tensor_scalar(
                                            out=pt.bitcast(i16)[:, csl], in0=ps_s[hp][:, csl],
                                            scalar1=EXP_A, scalar2=EXP_B,
                                            op0=OP.mult, op1=OP.add)
                                    else:
                                        nc.scalar.activation(out=pt[:, csl],
                                                             in_=ps_s[hp][:, csl],
                                                             func=AF.Exp, scale=0.125)
                                pts[hp] = pt
                            for icc in range(2):
                                for hp in range(2):
                                    nc.tensor.matmul(
                                        ps_o[hp][:, icc * 512:(icc + 1) * 512],
                                        v_all[:, jt, 2 * pair + hp, :],
                                        pts[hp][:, icc * 512:(icc + 1) * 512],
                                        start=(jt == 0), stop=(jt == 15))

                        for hp in range(2):
                            h = 2 * pair + hp
                            slot = 2 * h + ih
                            z_sb = nrm.tile([1, IH], f32, tag="z_sb", name=f"z{slot}")
                            nc.scalar.activation(out=z_sb, in_=ps_o[hp][64:65, :], func=AF.Copy)
                            nc.sync.dma_start(out=z_d[slot:slot + 1, :], in_=z_sb)
                            zt = nrm.tile([128, 8], f32, tag="zt", name=f"zt{slot}")
                            nc.sync.dma_start(out=zt,
                                              in_=z_d[slot, :].rearrange("(p t) -> p t", p=128))
                            rt = nrm.tile([128, 8], f32, tag="rt", name=f"rt{slot}")
                            nc.vector.reciprocal(out=rt, in_=zt)
                            rt_bf = nrm.tile([128, 8], bf16, tag="rt_bf", name=f"rtb{slot}")
                            nc.vector.tensor_copy(out=rt_bf, in_=rt)
                            nc.sync.dma_start(out=zr_d[slot, :].rearrange("(p t) -> p t", p=128),
                                              in_=rt_bf)
                            r64 = nrm.tile([64, IH], bf16, tag="r64", name=f"r64_{slot}")
                            nc.sync.dma_start(out=r64, in_=row_bcast(zr_d[slot, :], 64, IH))
                            outT_t = nrm.tile([64, IH], bf16, tag="outT", name=f"oT{slot}")
                            nc.vector.tensor_tensor(out=outT_t, in0=ps_o[hp][0:64, :],
                                                    in1=r64, op=OP.mult)
                            nc.sync.dma_start(
                                out=cc_in[pair][4 * ih:4 * ih + 4,
                                                hp * 64:(hp + 1) * 64, :]
                                    .rearrange("s d i -> d s i"),
                                in_=outT_t.rearrange("d (s i) -> d s i", s=4))

                    nc.gpsimd.collective_compute(
                        "AllToAll", mybir.AluOpType.bypass, replica_groups=groups,
                        ins=[cc_in[pair].opt()], outs=[cc_out[pair].opt()])
                    emit_at_loads(pair)

            with ExitStack() as cctx:
                psP = cctx.enter_context(tc.tile_pool(name="psP", bufs=1, space="PSUM"))
                oup = cctx.enter_context(tc.tile_pool(name="out_pool", bufs=3))

                ps_list = {}
                for bb in range(B):
                    for mt in range(2):
                        for nk in range(2):
                            ps_p = psP.tile([128, 512], f32, tag=f"ps_p{bb}{mt}{nk}")
                            ps_list[(bb, mt, nk)] = ps_p
                kc_order = [0, 2, 4, 6, 1, 3, 5, 7]
                for ki, kc in enumerate(kc_order):
                    wp_t = wp_sb[:, kc, :]
                    for bb in range(B):
                        at_t = at_tiles[kc]
                        for mt in range(2):
                            for nk in range(2):
                                nc.tensor.matmul(
                                    ps_list[(bb, mt, nk)],
                                    at_t[:, bb, mt * 128:(mt + 1) * 128],
                                    wp_t[:, nk * 512:(nk + 1) * 512],
                                    start=(ki == 0), stop=(ki == 7))
                for bb in range(B):
                    for mt in range(2):
                        o_sb = oup.tile([128, C], f32, tag="o_sb")
                        for nk in range(2):
                            nc.vector.tensor_tensor(
                                out=o_sb[:, nk * 512:(nk + 1) * 512],
                                in0=ps_list[(bb, mt, nk)],
                                in1=bproj_bc[:, nk * 512:(nk + 1) * 512],
                                op=OP.add)
                        nc.sync.dma_start(
                            out=out_d.ap()[bb, mt * 128:(mt + 1) * 128, :], in_=o_sb)

    nc.compile()
    return nc


def kernel(**inputs):
    from concourse.bass_utils import run_bass_kernel_spmd
    import ml_dtypes

    trace = os.environ.get("KERNEL_TRACE", "0") == "1"
    if trace:
        _install_trace_shim()

    bf = ml_dtypes.bfloat16

    x = np.asarray(inputs["x"], dtype=np.float32)
    w_qkv = np.asarray(inputs["w_qkv"], dtype=np.float32)
    b_qkv = np.asarray(inputs["b_qkv"], dtype=np.float32)
    w_proj = np.asarray(inputs["w_proj"], dtype=np.float32)
    b_proj = np.asarray(inputs["b_proj"], dtype=np.float32)
    q_scale = np.asarray(inputs["q_scale"], dtype=np.float32)
    q_bias = np.asarray(inputs["q_bias"], dtype=np.float32)
    k_scale = np.asarray(inputs["k_scale"], dtype=np.float32)
    k_bias = np.asarray(inputs["k_bias"], dtype=np.float32)

    general = not (np.all(q_scale == 1.0) and np.all(k_scale == 1.0)
                   and np.all(q_bias == 0.0) and np.all(k_bias == 0.0))

    key = "nc_gen" if general else "nc_fast"
    if key not in _CACHE:
        _CACHE[key] = _build(general)
    nc = _CACHE[key]

    L = np.zeros((128, 2), dtype=np.float32)
    L[0:64, 0] = 1.0 / 64.0
    L[64:128, 1] = 1.0 / 64.0

    wproj_m = np.ascontiguousarray(w_proj.astype(bf))

    in_maps = []
    for c in range(N_CORES):
        b, r = divmod(c, 4)
        base = 4 * r * D
        wq = np.ascontiguousarray(
            w_qkv[:, 0 * C + base: 0 * C + base + 256].reshape(C, 2, 128).astype(bf))
        wk = np.ascontiguousarray(
            w_qkv[:, 1 * C + base: 1 * C + base + 256].reshape(C, 2, 128).astype(bf))
        wv = np.ascontiguousarray(w_qkv[:, 2 * C + base: 2 * C + base + 256].astype(bf))
        bq = np.ascontiguousarray(b_qkv[0 * C + base: 0 * C + base + 256].reshape(2, 128))
        bk = np.ascontiguousarray(b_qkv[1 * C + base: 1 * C + base + 256].reshape(2, 128))
        bv = np.ascontiguousarray(b_qkv[2 * C + base: 2 * C + base + 256])
        m = {
            "xT": np.ascontiguousarray(x[b].T.astype(bf)),
            "wq": wq, "wk": wk, "wv": wv, "wproj": wproj_m,
            "bq": bq, "bk": bk, "bv": bv, "bproj": b_proj,
            "lnL": np.ascontiguousarray(L.astype(bf)),
        }
        if general:
            m["gq"] = np.ascontiguousarray(np.tile(q_scale, 2).reshape(2, 128))
            m["gk"] = np.ascontiguousarray(np.tile(k_scale, 2).reshape(2, 128))
            m["hq"] = np.ascontiguousarray(np.tile(q_bias, 2).reshape(2, 128))
            m["hk"] = np.ascontiguousarray(np.tile(k_bias, 2).reshape(2, 128))
        else:
            z2 = np.zeros((2, 128), dtype=np.float32)
            m["gq"] = z2; m["gk"] = z2; m["hq"] = z2; m["hk"] = z2
        in_maps.append(m)

    res = run_bass_kernel_spmd(nc, in_maps, core_ids=list(range(N_CORES)),
                               trace=trace)
    _CACHE["last_result"] = res

    out = np.empty((B, N, C), dtype=np.float32)
    for c in range(N_CORES):
        out[:, c * 256:(c + 1) * 256, :] = res.results[c]["out_part"]
    return out
